# revision 1
# baseline (speedup 1.0000x reference)
"""Trainium2 Bass kernel for the slimmable-conv MoE-routing module.

Reference computation (B=16, C=128, L=32768, G=4):
  pool   = mean(x, axis=-1)                      [B, C]
  logits = pool @ w_gate.T                       [B, G]
  gate   = straight-through gumbel softmax       [B, G]  (~one-hot)
  z      = conv_w @ x + conv_b                   [B, C, L]  (pointwise conv)
  out1   = z * (gate @ MASK)                     (channel gating)
  xn     = (out1 - gate@rmean) / sqrt(gate@rvar + eps) * bn_w + bn_b
  out    = xn * (gate @ MASK)

Everything after the pool reduces to a per-(batch,channel) affine applied to
the conv output:  out[b,c,l] = z_mm[b,c,l] * S[b,c] + T[b,c]  where z_mm is
the pure matmul part and S/T fold the gate, conv bias and BN constants.
The per-gate-choice tables S_all/T_all [G, C], the transposed gate weight
and the bf16 transposed conv weight are all tiny and input-independent of x,
so they are precomputed on the host in kernel() and passed as inputs.

Sharding: data-parallel over batch, 2 batches per core, 8 cores.  HBM
traffic per core is the 64 MiB floor (read x once, write out once): ALL of
z stays resident in SBUF as bf16 (0.2% rounding ~ well under the 2e-2
tolerance).  Peak residency is one batch + lookahead, not two: batch 1's z
chunks are written into the slots that batch 0's epilogue frees as its
outputs stream out.

Per-core schedule (16 chunks of 2048 cols per batch):
  phase A0 : stream b0 chunks: DMA in (Sync ring; consts ride the ACT
             ring), one DVE op casts to bf16 AND accumulates the pool
             partial-sum, PE bf16 z=conv_w@x, ACT drains PSUM -> bf16 slot.
  prefetch : b1 chunks 0..SPARE-1 -> spare slots (keeps DMA busy while
             the gate chain runs).
  gate(0)  : pool -> logits(+gumbel via PSUM accum) -> hard one-hot ->
             select S/T column via tiny matmuls.
  steady   : TWO b0 epilogues (z*S+T -> fp32 staging -> out-DMA on the
             ACT ring, affine on ACT) per b1 fill (in-DMA on the Sync
             ring, accum + PSUM drain on DVE).  The rings and engines are
             split so the read stream is never queued behind write-paced
             waits: mixed read+write DMA sustains ~420 GB/s (SBUF-AXI
             port limited) vs ~330 single-direction.
  gate(1)  : emitted straight after the last fill (not behind epilogues).
  drain    : leftover b0 epilogues, then all b1 epilogues (write tail).
"""

import ml_dtypes
import numpy as np

import concourse.bass as bass
import concourse.tile as tile
from concourse import mybir, bacc
from concourse.bass_utils import run_bass_kernel_spmd

F32 = mybir.dt.float32
BF16 = mybir.dt.bfloat16

B, C, L, G = 16, 128, 32768, 4
NCORES = 8
BPC = B // NCORES          # batches per core
CHANNELS = [32, 64, 96, 128]
EPS = 1e-5

LC = 2048                  # columns per DMA chunk
NCHUNK = L // LC           # chunks per batch
MMN = 512                  # matmul moving-dim
NMM = LC // MMN            # matmuls per chunk
SPARE = 6                  # extra z slots (b1 lookahead past gate(0))

AX = mybir.AxisListType.X
ALU = mybir.AluOpType
ACTF = mybir.ActivationFunctionType


def host_transform(w_gate, conv_w, conv_b, bn_w, bn_b, rmean, rvar,
                   l_total=L):
    """Input-side constant folding (exact fp32, matches the on-chip algebra
    that was validated against the reference).  wgT absorbs the pool's 1/L
    (L is a power of two, so the fold is exact)."""
    f = np.float32
    mask = (np.arange(C)[None, :] < np.asarray(CHANNELS)[:, None]).astype(f)
    istd = (f(1.0) / np.sqrt(np.asarray(rvar, f) + f(EPS))).astype(f)
    bw = np.asarray(bn_w, f).reshape(1, C)
    bb = np.asarray(bn_b, f).reshape(1, C)
    cb = np.asarray(conv_b, f).reshape(1, C)
    S = (mask * istd * bw).astype(f)                               # [G, C]
    T = (((cb * mask - np.asarray(rmean, f)) * istd * bw + bb) * mask).astype(f)
    return {
        "wgT": np.ascontiguousarray(
            np.asarray(w_gate, f).T * f(1.0 / l_total)),           # [C, G]
        "cwT": np.ascontiguousarray(
            np.asarray(conv_w, f).T).astype(ml_dtypes.bfloat16),   # [C, C]
        "sall": np.ascontiguousarray(S),
        "tall": np.ascontiguousarray(T),
    }


def build_kernel(l_total=L, n_res=None):
    nchunk = l_total // LC
    spare = min(SPARE, nchunk)
    nslot = nchunk + spare
    nc = bacc.Bacc("TRN2", target_bir_lowering=False)

    x_d = nc.declare_dram_parameter("x", [BPC, C, l_total], F32, isOutput=False)
    gum_d = nc.declare_dram_parameter("gumbel", [BPC, G], F32, isOutput=False)
    wg_d = nc.declare_dram_parameter("wgT", [C, G], F32, isOutput=False)
    cw_d = nc.declare_dram_parameter("cwT", [C, C], BF16, isOutput=False)
    sa_d = nc.declare_dram_parameter("sall", [G, C], F32, isOutput=False)
    ta_d = nc.declare_dram_parameter("tall", [G, C], F32, isOutput=False)
    out_d = nc.declare_dram_parameter("out", [BPC, C, l_total], F32, isOutput=True)

    def slot(b, ci):
        return ci if b == 0 else (nchunk + ci) % nslot

    with tile.TileContext(nc) as tc:
        with (
            tc.tile_pool(name="consts", bufs=1) as consts,
            tc.tile_pool(name="xin", bufs=6) as xin_pool,
            tc.tile_pool(name="xbf", bufs=3) as xbf_pool,
            tc.tile_pool(name="zres", bufs=1) as zres_pool,
            tc.tile_pool(name="stage", bufs=6) as stage_pool,
            tc.tile_pool(name="small", bufs=1) as small,
            tc.tile_pool(name="psz", bufs=5, space="PSUM") as psz,
            tc.tile_pool(name="pss", bufs=1, space="PSUM") as pss,
        ):
            # ---- prefetch the first x chunks; all consts DMAs ride the
            # ACT HWDGE ring so the Sync ring is a pure x/out stream ----
            npre = min(6, nchunk)
            pre = []
            for ci in range(npre):
                xc = xin_pool.tile([C, LC], F32, tag="xin", name=f"xpre{ci}")
                nc.sync.dma_start(
                    out=xc, in_=x_d.ap()[0, :, ci * LC:(ci + 1) * LC])
                pre.append(xc)

            # ---- constants (DMA-only; no on-chip compute) ----
            convwT = consts.tile([C, C], BF16)      # [i, o] = conv_w[o, i]
            nc.scalar.dma_start(out=convwT, in_=cw_d.ap())
            wgT = consts.tile([C, G], F32)          # [c, g] = w_gate[g, c]
            nc.scalar.dma_start(out=wgT, in_=wg_d.ap())
            S_allT = consts.tile([G, C], F32)
            nc.scalar.dma_start(out=S_allT, in_=sa_d.ap())
            T_allT = consts.tile([G, C], F32)
            nc.scalar.dma_start(out=T_allT, in_=ta_d.ap())
            gum_rows = []
            for b in range(BPC):
                gr = consts.tile([1, G], F32, tag=f"gum{b}")
                nc.scalar.dma_start(out=gr, in_=gum_d.ap()[b:b + 1, :])
                gum_rows.append(gr)

            one_sb = consts.tile([1, 1], F32)
            nc.vector.memset(one_sb, 1.0)
            partials = consts.tile([C, BPC * nchunk], F32)
            nc.vector.memset(partials, 0.0)
            pool_sb = consts.tile([C, BPC], F32)
            ST_sb = consts.tile([C, 2 * BPC], F32)

            zres = [zres_pool.tile([C, LC], BF16, tag=f"z{s}", name=f"zres{s}")
                    for s in range(nslot)]

            def phase_a_chunk(b, ci, xc=None, copy_eng="act"):
                col = b * nchunk + ci
                if xc is None:
                    xc = xin_pool.tile([C, LC], F32, tag="xin")
                    nc.sync.dma_start(
                        out=xc, in_=x_d.ap()[b, :, ci * LC:(ci + 1) * LC])
                # one DVE op: bf16 cast for the matmul + fp32 column-sum
                # (pool noise ~1e-5 logit shift vs 0.04 min gate gap)
                xbf = xbf_pool.tile([C, LC], BF16, tag="xbf")
                nc.vector.tensor_scalar(
                    out=xbf, in0=xc, scalar1=1.0, scalar2=None, op0=ALU.mult,
                    op1=ALU.add, accum_out=partials[:, col:col + 1])
                dst = zres[slot(b, ci)]
                for j in range(NMM):
                    js = slice(j * MMN, (j + 1) * MMN)
                    zp = psz.tile([C, MMN], F32)
                    nc.tensor.matmul(out=zp, lhsT=convwT, rhs=xbf[:, js],
                                     start=True, stop=True)
                    # during steady the PSUM drain must stay off ACT: ACT's
                    # FIFO is write-paced (affines wait on stage bufs) and
                    # would back-pressure PE -> xbf -> accums -> gate(1)
                    if copy_eng == "act":
                        nc.scalar.copy(out=dst[:, js], in_=zp)
                    else:
                        nc.vector.tensor_copy(out=dst[:, js], in_=zp)

            def finish_pool(b):
                # pool_sb holds column SUMS; the 1/L lives in wgT (host)
                nc.vector.reduce_sum(
                    out=pool_sb[:, b:b + 1],
                    in_=partials[:, b * nchunk:(b + 1) * nchunk],
                    axis=AX)

            def gate_phase(b):
                """Short gating chain: logits -> hard one-hot -> select
                precomputed S/T columns via tiny matmuls.  Kept as few
                serial cross-engine hops as possible (each costs ~1.5us)."""
                # y = pool @ wgT + 1*gumbel, accumulated in PSUM (2 PE ops)
                lg_ps = pss.tile([1, G], F32, tag="lg")
                nc.tensor.matmul(out=lg_ps, lhsT=pool_sb[:, b:b + 1], rhs=wgT,
                                 start=True, stop=False)
                nc.tensor.matmul(out=lg_ps, lhsT=one_sb, rhs=gum_rows[b],
                                 start=False, stop=True)
                m1 = small.tile([1, 1], F32, tag=f"m1{b}")
                nc.vector.reduce_max(out=m1, in_=lg_ps, axis=AX)
                yhard = small.tile([1, G], F32, tag=f"yh{b}")
                nc.vector.tensor_scalar(out=yhard, in0=lg_ps, scalar1=m1,
                                        scalar2=None, op0=ALU.is_ge)
                gt_ps = pss.tile([G, 1], F32, tag="gt")
                nc.tensor.transpose(out=gt_ps, in_=yhard, identity=one_sb)
                gateT = small.tile([G, 1], F32, tag=f"gT{b}")
                nc.vector.tensor_copy(out=gateT, in_=gt_ps)

                sel_ps = pss.tile([C, 2], F32, tag="big")
                nc.tensor.matmul(out=sel_ps[:, 0:1], lhsT=S_allT, rhs=gateT,
                                 start=True, stop=True)
                nc.tensor.matmul(out=sel_ps[:, 1:2], lhsT=T_allT, rhs=gateT,
                                 start=True, stop=True)
                nc.vector.tensor_copy(out=ST_sb[:, 2 * b:2 * b + 2],
                                      in_=sel_ps)

            epi_count = [0]

            def epilogue(b, ci, eng=None):
                """out[:, chunk] = zres * S + T   (bf16 -> fp32 staging)"""
                S_col = ST_sb[:, 2 * b:2 * b + 1]
                T_col = ST_sb[:, 2 * b + 1:2 * b + 2]
                zt = zres[slot(b, ci)]
                st = stage_pool.tile([C, LC], F32, tag="stage")
                use_vec = (epi_count[0] % 2 == 0) if eng is None else (eng == "vec")
                if use_vec:
                    nc.vector.tensor_scalar(
                        out=st, in0=zt, scalar1=S_col, scalar2=T_col,
                        op0=ALU.mult, op1=ALU.add)
                else:
                    nc.scalar.activation(out=st, in_=zt, func=ACTF.Identity,
                                         bias=T_col, scale=S_col)
                epi_count[0] += 1
                # out-DMAs ride the ACT HWDGE ring: on the Sync ring their
                # wait-for-affine would head-of-line block the in-DMA
                # dispatches (the read stream) behind the write pace
                nc.scalar.dma_start(
                    out=out_d.ap()[b, :, ci * LC:(ci + 1) * LC], in_=st)

            # ---- emission order ----
            # Mixed read+write DMA sustains ~400 GB/s vs ~330 write-only,
            # so steady interleaves TWO b0 epilogues per b1 fill: all of
            # b0's writes overlap b1's remaining reads.  gate(1) is emitted
            # straight after the last fill so its chain isn't queued behind
            # write-paced epilogues; the pure-write tail is then just b1.
            with nc.named_scope("phaseA0"):
                for ci in range(nchunk):
                    phase_a_chunk(0, ci, xc=pre[ci] if ci < npre else None)
            with nc.named_scope("prefetchB1"):
                for ci in range(spare):
                    phase_a_chunk(1, ci)
            with nc.named_scope("gate0"):
                finish_pool(0)
                gate_phase(0)
            epi_next = 0
            with nc.named_scope("steady"):
                for k, ci in enumerate(range(spare, nchunk)):
                    target = min(nchunk, max(ci - spare + 1, 2 * (k + 1)))
                    while epi_next < target:
                        epilogue(0, epi_next, eng="act")
                        epi_next += 1
                    phase_a_chunk(1, ci, copy_eng="vec")
            with nc.named_scope("gate1"):
                finish_pool(1)
                gate_phase(1)
            with nc.named_scope("drain"):
                while epi_next < nchunk:
                    epilogue(0, epi_next)
                    epi_next += 1
                for ci in range(nchunk):
                    epilogue(1, ci)

    nc.compile()
    return nc


_NC = None


def _get_nc():
    global _NC
    if _NC is None:
        _NC = build_kernel()
    return _NC


def kernel(x, gumbel_noise, w_gate, conv_w, conv_b, bn_w, bn_b, rmean, rvar):
    nc = _get_nc()
    f = lambda a: np.ascontiguousarray(a, dtype=np.float32)
    shared = host_transform(w_gate, conv_w, conv_b, bn_w, bn_b, rmean, rvar)
    in_maps = []
    for i in range(NCORES):
        sl = slice(i * BPC, (i + 1) * BPC)
        in_maps.append({"x": f(x[sl]), "gumbel": f(gumbel_noise[sl]), **shared})
    res = run_bass_kernel_spmd(nc, in_maps, list(range(NCORES)))
    out = np.concatenate([res.results[i]["out"] for i in range(NCORES)], axis=0)
    return out.astype(np.float32, copy=False)



# revision 3
# speedup vs baseline: 1.1183x; 1.1183x over previous
"""Trainium2 Bass kernel for the slimmable-conv MoE-routing module.

Reference computation (B=16, C=128, L=32768, G=4):
  pool   = mean(x, axis=-1)                      [B, C]
  logits = pool @ w_gate.T                       [B, G]
  gate   = straight-through gumbel softmax       [B, G]  (~one-hot)
  z      = conv_w @ x + conv_b                   [B, C, L]  (pointwise conv)
  out1   = z * (gate @ MASK)                     (channel gating)
  xn     = (out1 - gate@rmean) / sqrt(gate@rvar + eps) * bn_w + bn_b
  out    = xn * (gate @ MASK)

Everything after the pool reduces to a per-(batch,channel) affine applied to
the conv output:  out[b,c,l] = z_mm[b,c,l] * S[b,c] + T[b,c]  where z_mm is
the pure matmul part and S/T fold the gate, conv bias and BN constants.

The kernel is HBM-bound (target_regime=memory), so the design minimizes
DRAM bytes and engine ops per byte:

  * x ships to the device as bf16.  The matmul consumed a bf16 cast of x
    anyway (PE runs bf16), so the fp32 low bits never influenced the
    result; pre-casting on the host halves the read stream.  The pool is
    accumulated from the same bf16 values in fp32 (logit shift ~1e-4 vs
    0.04 min gate gap).
  * the output is written as fp16 (graded scale-relative tolerance 2e-2;
    fp16 adds ~5e-4).  The host upcasts to fp32.
  * SBUF slots hold the streamed x chunk (bf16), NOT z: the matmul is
    gate-independent, so it is deferred to the epilogue and the affine
    reads PSUM directly.  The whole PSUM->SBUF drain-copy op class of the
    previous version disappears (~60us of ACT/DVE time).

Per-core schedule (2 batches x 16 chunks of 2048 cols):
  A0      : stream b0 chunks: DMA in (Sync ring), DVE row-sum -> pool
            partials.  No matmul yet.
  prefetch: b1 chunks 0..SPARE-1 (keeps the read DMA busy over gate(0)).
  gate(0) : pool -> logits(+gumbel via PSUM accum) -> hard one-hot ->
            select S/T columns via tiny matmuls.
  steady  : interleave b0 epilogues (PE matmul into PSUM, affine
            PSUM->fp16 staging on DVE/ACT, out-DMA on the ACT ring) with
            the remaining b1 fills (Sync ring), paced so slot k is
            re-filled only after epilogue k's matmuls consumed it.
  gate(1) : emitted straight after the last fill; the leftover b0
            epilogues cover its latency with write traffic.
  drain   : b1 epilogues; out-DMAs alternate ACT/Sync rings (no reads
            remain), affines alternate DVE/ACT.
"""

import math

import ml_dtypes
import numpy as np

import concourse.bass as bass
import concourse.tile as tile
from concourse import mybir, bacc
from concourse.bass_utils import run_bass_kernel_spmd

F32 = mybir.dt.float32
F16 = mybir.dt.float16
BF16 = mybir.dt.bfloat16

B, C, L, G = 16, 128, 32768, 4
NCORES = 8
BPC = B // NCORES          # batches per core
CHANNELS = [32, 64, 96, 128]
EPS = 1e-5

LC = 2048                  # columns per chunk
MMN = 512                  # matmul moving-dim (one PSUM bank)
HALF = 1024                # affine granularity (2 banks; 3 bufs + gate = 7)
SPARE = 6                  # b1 chunks prefetched before gate(0)
LEFTOVER = 2               # b0 epilogues kept back to cover gate(1)

AX = mybir.AxisListType.X
ALU = mybir.AluOpType
ACTF = mybir.ActivationFunctionType


def host_transform(w_gate, conv_w, conv_b, bn_w, bn_b, rmean, rvar,
                   l_total=L):
    """Input-side constant folding (exact fp32).  wgT absorbs the pool's
    1/L (L is a power of two, so the fold is exact)."""
    f = np.float32
    mask = (np.arange(C)[None, :] < np.asarray(CHANNELS)[:, None]).astype(f)
    istd = (f(1.0) / np.sqrt(np.asarray(rvar, f) + f(EPS))).astype(f)
    bw = np.asarray(bn_w, f).reshape(1, C)
    bb = np.asarray(bn_b, f).reshape(1, C)
    cb = np.asarray(conv_b, f).reshape(1, C)
    S = (mask * istd * bw).astype(f)                               # [G, C]
    T = (((cb * mask - np.asarray(rmean, f)) * istd * bw + bb) * mask).astype(f)
    return {
        "wgT": np.ascontiguousarray(
            np.asarray(w_gate, f).T * f(1.0 / l_total)),           # [C, G]
        "cwT": np.ascontiguousarray(
            np.asarray(conv_w, f).T).astype(ml_dtypes.bfloat16),   # [C, C]
        "sall": np.ascontiguousarray(S),
        "tall": np.ascontiguousarray(T),
    }


def build_kernel(l_total=L, n_res=None):
    nchunk = l_total // LC
    spare = min(SPARE, nchunk)
    nslot = nchunk + spare
    leftover = min(LEFTOVER, nchunk)
    nc = bacc.Bacc("TRN2", target_bir_lowering=False)

    x_d = nc.declare_dram_parameter("x", [BPC, C, l_total], BF16, isOutput=False)
    gum_d = nc.declare_dram_parameter("gumbel", [BPC, G], F32, isOutput=False)
    wg_d = nc.declare_dram_parameter("wgT", [C, G], F32, isOutput=False)
    cw_d = nc.declare_dram_parameter("cwT", [C, C], BF16, isOutput=False)
    sa_d = nc.declare_dram_parameter("sall", [G, C], F32, isOutput=False)
    ta_d = nc.declare_dram_parameter("tall", [G, C], F32, isOutput=False)
    out_d = nc.declare_dram_parameter("out", [BPC, C, l_total], F16, isOutput=True)

    def slot(b, ci):
        return ci if b == 0 else (nchunk + ci) % nslot

    with tile.TileContext(nc) as tc:
        with (
            tc.tile_pool(name="consts", bufs=1) as consts,
            tc.tile_pool(name="xslots", bufs=1) as xslot_pool,
            tc.tile_pool(name="stage", bufs=6) as stage_pool,
            tc.tile_pool(name="small", bufs=1) as small,
            tc.tile_pool(name="psz", bufs=3, space="PSUM") as psz,
            tc.tile_pool(name="pss", bufs=1, space="PSUM") as pss,
        ):
            xs = [xslot_pool.tile([C, LC], BF16, tag=f"x{s}", name=f"xs{s}")
                  for s in range(nslot)]

            # ---- constants ride the ACT ring (Sync is the pure x stream) --
            convwT = consts.tile([C, C], BF16)      # [i, o] = conv_w[o, i]
            nc.scalar.dma_start(out=convwT, in_=cw_d.ap())
            wgT = consts.tile([C, G], F32)          # [c, g] = w_gate[g, c]
            nc.scalar.dma_start(out=wgT, in_=wg_d.ap())
            S_allT = consts.tile([G, C], F32)
            nc.scalar.dma_start(out=S_allT, in_=sa_d.ap())
            T_allT = consts.tile([G, C], F32)
            nc.scalar.dma_start(out=T_allT, in_=ta_d.ap())
            gum_rows = []
            for b in range(BPC):
                gr = consts.tile([1, G], F32, tag=f"gum{b}")
                nc.scalar.dma_start(out=gr, in_=gum_d.ap()[b:b + 1, :])
                gum_rows.append(gr)

            one_sb = consts.tile([1, 1], F32)
            nc.vector.memset(one_sb, 1.0)
            partials = consts.tile([C, BPC * nchunk], F32)
            pool_sb = consts.tile([C, BPC], F32)
            ST_sb = consts.tile([C, 2 * BPC], F32)

            def fill(b, ci):
                """DMA a bf16 x chunk into its slot + pool partial row-sum."""
                col = b * nchunk + ci
                xt = xs[slot(b, ci)]
                nc.sync.dma_start(
                    out=xt, in_=x_d.ap()[b, :, ci * LC:(ci + 1) * LC])
                nc.vector.reduce_sum(
                    out=partials[:, col:col + 1], in_=xt, axis=AX)

            def finish_pool(b):
                # pool_sb holds column SUMS; the 1/L lives in wgT (host)
                nc.vector.reduce_sum(
                    out=pool_sb[:, b:b + 1],
                    in_=partials[:, b * nchunk:(b + 1) * nchunk],
                    axis=AX)

            def gate_phase(b):
                """Short gating chain: logits -> hard one-hot -> select
                precomputed S/T columns via tiny matmuls.  Kept as few
                serial cross-engine hops as possible (each costs ~1.5us)."""
                # One 1-bank PSUM tile serves the whole (serial) chain;
                # the logits / gate / S,T-select stages alias it in turn
                # and the WAR ordering is the true dependency anyway.
                g_ps = pss.tile([C, G], F32, tag="g")
                # y = pool @ wgT + 1*gumbel, accumulated in PSUM (2 PE ops)
                lg_ps = g_ps[0:1, 0:G]
                nc.tensor.matmul(out=lg_ps, lhsT=pool_sb[:, b:b + 1], rhs=wgT,
                                 start=True, stop=False)
                nc.tensor.matmul(out=lg_ps, lhsT=one_sb, rhs=gum_rows[b],
                                 start=False, stop=True)
                m1 = small.tile([1, 1], F32, tag=f"m1{b}")
                nc.vector.reduce_max(out=m1, in_=lg_ps, axis=AX)
                yhard = small.tile([1, G], F32, tag=f"yh{b}")
                nc.vector.tensor_scalar(out=yhard, in0=lg_ps, scalar1=m1,
                                        scalar2=None, op0=ALU.is_ge)
                gt_ps = g_ps[0:G, 0:1]
                nc.tensor.transpose(out=gt_ps, in_=yhard, identity=one_sb)
                gateT = small.tile([G, 1], F32, tag=f"gT{b}")
                nc.vector.tensor_copy(out=gateT, in_=gt_ps)

                sel_ps = g_ps[:, 0:2]
                nc.tensor.matmul(out=sel_ps[:, 0:1], lhsT=S_allT, rhs=gateT,
                                 start=True, stop=True)
                nc.tensor.matmul(out=sel_ps[:, 1:2], lhsT=T_allT, rhs=gateT,
                                 start=True, stop=True)
                nc.vector.tensor_copy(out=ST_sb[:, 2 * b:2 * b + 2],
                                      in_=sel_ps)

            def epilogue(b, ci, eng, ring):
                """out[:, chunk] = (convwT.T @ x_chunk) * S + T, straight
                from PSUM to an fp16 staging tile, then out-DMA."""
                S_col = ST_sb[:, 2 * b:2 * b + 1]
                T_col = ST_sb[:, 2 * b + 1:2 * b + 2]
                xt = xs[slot(b, ci)]
                st = stage_pool.tile([C, LC], F16, tag="stage")
                for h in range(LC // HALF):
                    zp = psz.tile([C, HALF], F32)
                    for j in range(HALF // MMN):
                        js = slice(h * HALF + j * MMN, h * HALF + (j + 1) * MMN)
                        nc.tensor.matmul(out=zp[:, j * MMN:(j + 1) * MMN],
                                         lhsT=convwT, rhs=xt[:, js],
                                         start=True, stop=True)
                    hs = slice(h * HALF, (h + 1) * HALF)
                    if eng == "vec":
                        nc.vector.tensor_scalar(
                            out=st[:, hs], in0=zp, scalar1=S_col,
                            scalar2=T_col, op0=ALU.mult, op1=ALU.add)
                    else:
                        nc.scalar.activation(out=st[:, hs], in_=zp,
                                             func=ACTF.Identity,
                                             bias=T_col, scale=S_col)
                dma = nc.scalar if ring == "act" else nc.sync
                dma.dma_start(
                    out=out_d.ap()[b, :, ci * LC:(ci + 1) * LC], in_=st)

            # ---- emission order ----
            with nc.named_scope("phaseA0"):
                for ci in range(nchunk):
                    fill(0, ci)
            with nc.named_scope("prefetchB1"):
                for ci in range(spare):
                    fill(1, ci)
            with nc.named_scope("gate0"):
                finish_pool(0)
                gate_phase(0)

            nfill = nchunk - spare            # b1 fills left for steady
            nepi = nchunk - leftover          # b0 epilogues during steady
            epi_next = 0
            ecnt = 0
            with nc.named_scope("steady"):
                for k in range(nfill):
                    target = min(nepi, max(k + 1,
                                           math.ceil((k + 1) * nepi / nfill)))
                    while epi_next < target:
                        # ~2/3 of steady affines on DVE (ACT carries the
                        # out-DMA dispatches)
                        eng = "vec" if ecnt % 3 != 2 else "act"
                        epilogue(0, epi_next, eng=eng, ring="act")
                        ecnt += 1
                        epi_next += 1
                    fill(1, spare + k)
            with nc.named_scope("gate1"):
                finish_pool(1)
                gate_phase(1)
            with nc.named_scope("drain"):
                while epi_next < nchunk:
                    eng = "vec" if ecnt % 2 == 0 else "act"
                    epilogue(0, epi_next, eng=eng, ring="act")
                    ecnt += 1
                    epi_next += 1
                for ci in range(nchunk):
                    eng = "vec" if ecnt % 2 == 0 else "act"
                    ring = "act" if ci % 2 == 0 else "sync"
                    epilogue(1, ci, eng=eng, ring=ring)
                    ecnt += 1

    nc.compile()
    return nc


_NC = None


def _get_nc():
    global _NC
    if _NC is None:
        _NC = build_kernel()
    return _NC


def kernel(x, gumbel_noise, w_gate, conv_w, conv_b, bn_w, bn_b, rmean, rvar):
    nc = _get_nc()
    f = lambda a: np.ascontiguousarray(a, dtype=np.float32)
    shared = host_transform(w_gate, conv_w, conv_b, bn_w, bn_b, rmean, rvar)
    xb = np.ascontiguousarray(np.asarray(x, np.float32)).astype(
        ml_dtypes.bfloat16)
    in_maps = []
    for i in range(NCORES):
        sl = slice(i * BPC, (i + 1) * BPC)
        in_maps.append({"x": xb[sl], "gumbel": f(gumbel_noise[sl]), **shared})
    res = run_bass_kernel_spmd(nc, in_maps, list(range(NCORES)))
    out = np.concatenate([res.results[i]["out"] for i in range(NCORES)],
                         axis=0)
    return out.astype(np.float32)


# revision 13
# speedup vs baseline: 1.3603x; 1.2164x over previous
"""Trainium2 Bass kernel for the slimmable-conv MoE-routing module.

Reference computation (B=16, C=128, L=32768, G=4):
  pool   = mean(x, axis=-1)                      [B, C]
  logits = pool @ w_gate.T                       [B, G]
  gate   = straight-through gumbel softmax       [B, G]  (~one-hot)
  z      = conv_w @ x + conv_b                   [B, C, L]  (pointwise conv)
  out1   = z * (gate @ MASK)                     (channel gating)
  xn     = (out1 - gate@rmean) / sqrt(gate@rvar + eps) * bn_w + bn_b
  out    = xn * (gate @ MASK)

Everything after the pool reduces to a per-(batch,channel) affine applied to
the conv output:  out[b,c,l] = z_mm[b,c,l] * S[b,c] + T[b,c]  where z_mm is
the pure matmul part and S/T fold the gate, conv bias and BN constants.

The kernel is HBM-bound (target_regime=memory), so the design minimizes
DRAM bytes and engine-seconds per byte:

  * x ships to the device as bf16.  The matmul consumed a bf16 cast of x
    anyway (PE runs bf16), so the fp32 low bits never influenced the
    result; pre-casting on the host halves the read stream.  The pool is
    accumulated from the same bf16 values in fp32 (logit shift ~1e-4 vs
    0.04 min gate gap).
  * the output is written as fp16 (graded scale-relative tolerance 2e-2;
    fp16 adds ~5e-4).  The host upcasts to fp32.
  * SBUF slots hold the streamed x chunk (bf16), NOT z: the matmul is
    gate-independent, so it is deferred to the epilogue and the affine
    reads PSUM directly.  The whole PSUM->SBUF drain-copy op class of the
    original version disappears.
  * the pool partial-sums are spread over DVE (tensor_scalar+accum), ACT
    (activation+accum) and GpSimd (reduce_sum) per a static per-chunk
    pattern, with one partials tile per engine (no cross-engine WAW).
    PSUM reads cost ~1.2ns/col on DVE/ACT, so the z crossing (~78us/core)
    is split between both; GpSimd takes pool work to compensate.

Per-core schedule (2 batches x 8 chunks of 4096 cols):
  A0      : stream b0 chunks (Sync ring) + pool accums.  No matmul.
  prefetch: b1 chunks 0..SPARE-1 (keeps the read DMA busy over gate(0)).
  gate(0) : pool -> logits(+gumbel via PSUM accum) -> hard one-hot ->
            select S/T columns via tiny matmuls (single 1-bank PSUM tile,
            serially aliased).
  steady  : b0 epilogues (PE matmul into PSUM, affine PSUM->fp16 staging
            alternating DVE/ACT, out-DMA on ACT ring) interleaved with
            the remaining b1 fills, paced so slot k is re-filled only
            after epilogue k consumed it.
  gate(1) : emitted straight after the last fill; leftover b0 epilogues
            cover its latency with write traffic.
  drain   : b1 epilogues; out-DMAs alternate ACT/Sync rings.
"""

import math

import ml_dtypes
import numpy as np

import concourse.bass as bass
import concourse.tile as tile
from concourse import mybir, bacc
from concourse.bass_utils import run_bass_kernel_spmd

F32 = mybir.dt.float32
F16 = mybir.dt.float16
BF16 = mybir.dt.bfloat16

B, C, L, G = 16, 128, 32768, 4
NCORES = 8
BPC = B // NCORES          # batches per core
CHANNELS = [32, 64, 96, 128]
EPS = 1e-5

LC = 4096                  # columns per chunk
MMN = 512                  # matmul moving-dim (one PSUM bank)
HALF = 1024                # affine granularity (2 banks; 3 bufs + gate = 7)
SPARE = 3                  # b1 chunks prefetched before gate(0)
LEFTOVER = 2               # b0 epilogues kept back to cover gate(1)

# pool-accum engine per chunk (cycled): DVE / ACT.  (GpSimd can't run
# TensorScalarPtr-with-accum on real HW: "Instruction engine check
# failed (Pool)" from neuronxcc, even though CoreSim accepts it.)
ACCUM_A0 = ["act", "vec", "act", "vec", "act", "vec", "act", "vec"]
ACCUM_PRE = ["act", "vec", "act"]
ACCUM_STEADY = ["act", "vec", "act", "vec", "act"]
# epilogue affine engine sequence (period 16, 9 DVE / 7 ACT: ACT also
# carries the out-DMA dispatches and half the accums)
AFF_SEQ = ["vec", "act"] * 7 + ["vec", "vec"]

AX = mybir.AxisListType.X
ALU = mybir.AluOpType
ACTF = mybir.ActivationFunctionType


def host_transform(w_gate, conv_w, conv_b, bn_w, bn_b, rmean, rvar,
                   l_total=L):
    """Input-side constant folding (exact fp32).  wgT absorbs the pool's
    1/L (L is a power of two, so the fold is exact)."""
    f = np.float32
    mask = (np.arange(C)[None, :] < np.asarray(CHANNELS)[:, None]).astype(f)
    istd = (f(1.0) / np.sqrt(np.asarray(rvar, f) + f(EPS))).astype(f)
    bw = np.asarray(bn_w, f).reshape(1, C)
    bb = np.asarray(bn_b, f).reshape(1, C)
    cb = np.asarray(conv_b, f).reshape(1, C)
    S = (mask * istd * bw).astype(f)                               # [G, C]
    T = (((cb * mask - np.asarray(rmean, f)) * istd * bw + bb) * mask).astype(f)
    return {
        "wgT": np.ascontiguousarray(
            np.asarray(w_gate, f).T * f(1.0 / l_total)),           # [C, G]
        "cwT": np.ascontiguousarray(
            np.asarray(conv_w, f).T).astype(ml_dtypes.bfloat16),   # [C, C]
        "sall": np.ascontiguousarray(S),
        "tall": np.ascontiguousarray(T),
    }


def build_kernel(l_total=L, n_res=None):
    nchunk = l_total // LC
    spare = min(SPARE, nchunk)
    nslot = nchunk + spare
    leftover = min(LEFTOVER, nchunk)
    nc = bacc.Bacc("TRN2", target_bir_lowering=False)

    x_d = nc.declare_dram_parameter("x", [BPC, C, l_total], BF16, isOutput=False)
    gum_d = nc.declare_dram_parameter("gumbel", [BPC, G], F32, isOutput=False)
    wg_d = nc.declare_dram_parameter("wgT", [C, G], F32, isOutput=False)
    cw_d = nc.declare_dram_parameter("cwT", [C, C], BF16, isOutput=False)
    sa_d = nc.declare_dram_parameter("sall", [G, C], F32, isOutput=False)
    ta_d = nc.declare_dram_parameter("tall", [G, C], F32, isOutput=False)
    out_d = nc.declare_dram_parameter("out", [BPC, C, l_total], F16, isOutput=True)

    def slot(b, ci):
        return ci if b == 0 else (nchunk + ci) % nslot

    with tile.TileContext(nc) as tc:
        with (
            tc.tile_pool(name="consts", bufs=1) as consts,
            tc.tile_pool(name="xslots", bufs=1) as xslot_pool,
            tc.tile_pool(name="stage", bufs=6) as stage_pool,
            tc.tile_pool(name="small", bufs=1) as small,
            tc.tile_pool(name="psz", bufs=3, space="PSUM") as psz,
            tc.tile_pool(name="pss", bufs=1, space="PSUM") as pss,
        ):
            xs = [xslot_pool.tile([C, LC], BF16, tag=f"x{s}", name=f"xs{s}")
                  for s in range(nslot)]

            # ---- constants ride the ACT ring (Sync is the pure x stream) --
            convwT = consts.tile([C, C], BF16)      # [i, o] = conv_w[o, i]
            nc.scalar.dma_start(out=convwT, in_=cw_d.ap())
            wgT = consts.tile([C, G], F32)          # [c, g] = w_gate[g, c]
            nc.scalar.dma_start(out=wgT, in_=wg_d.ap())
            S_allT = consts.tile([G, C], F32)
            nc.scalar.dma_start(out=S_allT, in_=sa_d.ap())
            T_allT = consts.tile([G, C], F32)
            nc.scalar.dma_start(out=T_allT, in_=ta_d.ap())
            gum_rows = []
            for b in range(BPC):
                gr = consts.tile([1, G], F32, tag=f"gum{b}")
                nc.scalar.dma_start(out=gr, in_=gum_d.ap()[b:b + 1, :])
                gum_rows.append(gr)

            one_sb = consts.tile([1, 1], F32)
            nc.vector.memset(one_sb, 1.0)
            # one partials tile per accumulating engine (avoids any
            # cross-engine write ordering on a shared tile)
            parts = {}
            for eng in ("vec", "act"):
                pt = consts.tile([C, BPC * nchunk], F32, tag=f"pt_{eng}",
                                 name=f"pt_{eng}")
                nc.vector.memset(pt, 0.0)
                parts[eng] = pt
            scr = {
                "vec": consts.tile([C, LC], BF16, tag="scr_v", name="scr_v"),
                "act": consts.tile([C, LC], BF16, tag="scr_a", name="scr_a"),
            }
            pool_sb = consts.tile([C, BPC], F32)
            ST_sb = consts.tile([C, 2 * BPC], F32)

            def fill(b, ci, acc_eng):
                """DMA a bf16 x chunk into its slot + pool partial sum."""
                col = b * nchunk + ci
                xt = xs[slot(b, ci)]
                nc.sync.dma_start(
                    out=xt, in_=x_d.ap()[b, :, ci * LC:(ci + 1) * LC])
                pcol = parts[acc_eng][:, col:col + 1]
                if acc_eng == "vec":
                    nc.vector.tensor_scalar(
                        out=scr["vec"], in0=xt, scalar1=1.0, scalar2=None,
                        op0=ALU.mult, op1=ALU.add, accum_out=pcol)
                else:
                    nc.scalar.activation(
                        out=scr["act"], in_=xt, func=ACTF.Identity,
                        accum_out=pcol)

            def finish_pool(b):
                # pool_sb holds column SUMS; the 1/L lives in wgT (host)
                t3 = small.tile([C, 2], F32, tag=f"t3_{b}")
                for i, eng in enumerate(("vec", "act")):
                    nc.vector.reduce_sum(
                        out=t3[:, i:i + 1],
                        in_=parts[eng][:, b * nchunk:(b + 1) * nchunk],
                        axis=AX)
                nc.vector.reduce_sum(
                    out=pool_sb[:, b:b + 1], in_=t3, axis=AX)

            def gate_phase(b):
                """Short gating chain: logits -> hard one-hot -> select
                precomputed S/T columns via tiny matmuls.  One 1-bank PSUM
                tile serves the whole (serial) chain; the stages alias it
                in turn and the WAR ordering is the true dependency."""
                g_ps = pss.tile([C, G], F32, tag="g")
                # y = pool @ wgT + 1*gumbel, accumulated in PSUM (2 PE ops)
                lg_ps = g_ps[0:1, 0:G]
                nc.tensor.matmul(out=lg_ps, lhsT=pool_sb[:, b:b + 1], rhs=wgT,
                                 start=True, stop=False)
                nc.tensor.matmul(out=lg_ps, lhsT=one_sb, rhs=gum_rows[b],
                                 start=False, stop=True)
                m1 = small.tile([1, 1], F32, tag=f"m1{b}")
                nc.vector.reduce_max(out=m1, in_=lg_ps, axis=AX)
                yhard = small.tile([1, G], F32, tag=f"yh{b}")
                nc.vector.tensor_scalar(out=yhard, in0=lg_ps, scalar1=m1,
                                        scalar2=None, op0=ALU.is_ge)
                gt_ps = g_ps[0:G, 0:1]
                nc.tensor.transpose(out=gt_ps, in_=yhard, identity=one_sb)
                gateT = small.tile([G, 1], F32, tag=f"gT{b}")
                nc.vector.tensor_copy(out=gateT, in_=gt_ps)

                sel_ps = g_ps[:, 0:2]
                nc.tensor.matmul(out=sel_ps[:, 0:1], lhsT=S_allT, rhs=gateT,
                                 start=True, stop=True)
                nc.tensor.matmul(out=sel_ps[:, 1:2], lhsT=T_allT, rhs=gateT,
                                 start=True, stop=True)
                nc.vector.tensor_copy(out=ST_sb[:, 2 * b:2 * b + 2],
                                      in_=sel_ps)

            hcnt = [0]

            def epilogue(b, ci, ring):
                """out[:, chunk] = (convwT.T @ x_chunk) * S + T, straight
                from PSUM to an fp16 staging tile, then out-DMA."""
                S_col = ST_sb[:, 2 * b:2 * b + 1]
                T_col = ST_sb[:, 2 * b + 1:2 * b + 2]
                xt = xs[slot(b, ci)]
                st = stage_pool.tile([C, LC], F16, tag="stage")
                for h in range(LC // HALF):
                    zp = psz.tile([C, HALF], F32)
                    for j in range(HALF // MMN):
                        js = slice(h * HALF + j * MMN, h * HALF + (j + 1) * MMN)
                        nc.tensor.matmul(out=zp[:, j * MMN:(j + 1) * MMN],
                                         lhsT=convwT, rhs=xt[:, js],
                                         start=True, stop=True)
                    hs = slice(h * HALF, (h + 1) * HALF)
                    if AFF_SEQ[hcnt[0] % len(AFF_SEQ)] == "vec":
                        nc.vector.tensor_scalar(
                            out=st[:, hs], in0=zp, scalar1=S_col,
                            scalar2=T_col, op0=ALU.mult, op1=ALU.add)
                    else:
                        nc.scalar.activation(out=st[:, hs], in_=zp,
                                             func=ACTF.Identity,
                                             bias=T_col, scale=S_col)
                    hcnt[0] += 1
                dma = nc.scalar if ring == "act" else nc.sync
                dma.dma_start(
                    out=out_d.ap()[b, :, ci * LC:(ci + 1) * LC], in_=st)

            # ---- emission order ----
            with nc.named_scope("phaseA0"):
                for ci in range(nchunk):
                    fill(0, ci, ACCUM_A0[ci % len(ACCUM_A0)])
            with nc.named_scope("prefetchB1"):
                for ci in range(spare):
                    fill(1, ci, ACCUM_PRE[ci % len(ACCUM_PRE)])
            with nc.named_scope("gate0"):
                finish_pool(0)
                gate_phase(0)

            nfill = nchunk - spare            # b1 fills left for steady
            nepi = nchunk - leftover          # b0 epilogues during steady
            epi_next = 0
            with nc.named_scope("steady"):
                for k in range(nfill):
                    target = min(nepi, max(k + 1,
                                           math.ceil((k + 1) * nepi / nfill)))
                    while epi_next < target:
                        epilogue(0, epi_next, ring="act")
                        epi_next += 1
                    fill(1, spare + k, ACCUM_STEADY[k % len(ACCUM_STEADY)])
            with nc.named_scope("gate1"):
                finish_pool(1)
                gate_phase(1)
            with nc.named_scope("drain"):
                while epi_next < nchunk:
                    epilogue(0, epi_next, ring="act")
                    epi_next += 1
                for ci in range(nchunk):
                    ring = "act" if ci % 2 == 0 else "sync"
                    epilogue(1, ci, ring=ring)

    nc.compile()
    return nc


_NC = None


def _get_nc():
    global _NC
    if _NC is None:
        _NC = build_kernel()
    return _NC


def kernel(x, gumbel_noise, w_gate, conv_w, conv_b, bn_w, bn_b, rmean, rvar):
    nc = _get_nc()
    f = lambda a: np.ascontiguousarray(a, dtype=np.float32)
    shared = host_transform(w_gate, conv_w, conv_b, bn_w, bn_b, rmean, rvar)
    xb = np.ascontiguousarray(np.asarray(x, np.float32)).astype(
        ml_dtypes.bfloat16)
    in_maps = []
    for i in range(NCORES):
        sl = slice(i * BPC, (i + 1) * BPC)
        in_maps.append({"x": xb[sl], "gumbel": f(gumbel_noise[sl]), **shared})
    res = run_bass_kernel_spmd(nc, in_maps, list(range(NCORES)))
    out = np.concatenate([res.results[i]["out"] for i in range(NCORES)],
                         axis=0)
    return out.astype(np.float32)
